# revision 2
# baseline (speedup 1.0000x reference)
"""CrossAttention on 8 TRN2 NeuronCores (tensor-parallel over heads).

Reference computation (B=4, N=2048, DIM=1024, 16 heads, head_dim=64):
    qkv = x @ Wqkv.T + bqkv ; q, k = split(qkv)  (v unused)
    attn = softmax(q @ k.T * scale) ; out = attn @ split_heads(context)
    return merge_heads(out) @ Wout.T + bout

Sharding: core c owns heads {2c, 2c+1}. Each core computes q/k
projections for its heads (full sequence), head-parallel attention with
context slices as values, then an AllToAll re-shards from head-parallel
to row-parallel so the output projection runs locally on a 1024-row
slab. Output slabs are concatenated on the host.

All matmuls run in bf16 (fp32 PSUM accumulation); softmax runs exp on
ScalarE without max-subtraction (scores ~ N(0,1)), with the denominator
produced by an extra all-ones column appended to the value matrix.
"""
import numpy as np
import ml_dtypes

import concourse.bass as bass
import concourse.mybir as mybir
import concourse.tile as tile
from concourse import bacc
from concourse.bass_utils import run_bass_kernel_spmd

BF16 = ml_dtypes.bfloat16
F32 = mybir.dt.float32
BF = mybir.dt.bfloat16

NC = 8            # cores
B = 4             # batch
N = 2048          # sequence
DIM = 1024
NH = 16           # heads total
HD = 64           # head dim
HPC = NH // NC    # heads per core = 2
SCALE = HD ** -0.5
BN = B * N        # 8192 tokens
SLAB = BN // NC   # 1024 rows per core after re-shard
KC = DIM // 128   # contraction chunks for projections = 8
TC = BN // 512    # token chunks of 512 = 16
NKC = N // 128    # key chunks per batch = 16
CW = HD + 1       # value width incl. ones column = 65


def build():
    nc = bacc.Bacc("TRN2", target_bir_lowering=False, debug=False,
                   num_devices=NC)

    xT = nc.dram_tensor("xT", [DIM, BN], BF, kind="ExternalInput")
    wqkT = nc.dram_tensor("wqkT", [DIM, 2 * 128], BF, kind="ExternalInput")
    bqk = nc.dram_tensor("bqk", [2 * 128, 1], F32, kind="ExternalInput")
    ctxa = nc.dram_tensor("ctxa", [B, HPC, 128, NKC * CW], BF,
                          kind="ExternalInput")
    woutT = nc.dram_tensor("woutT", [DIM, DIM], BF, kind="ExternalInput")
    boutb = nc.dram_tensor("boutb", [128, DIM], F32, kind="ExternalInput")
    out = nc.dram_tensor("out", [SLAB, DIM], F32, kind="ExternalOutput")

    # AllToAll bounce buffers: [slab, feature, row] bf16
    a2a_in = nc.dram_tensor("a2a_in", [NC, 128, SLAB], BF)
    a2a_out = nc.dram_tensor("a2a_out", [NC, 128, SLAB], BF)

    with tile.TileContext(nc) as tc:
        with tc.tile_pool(name="const", bufs=1) as const, \
             tc.tile_pool(name="qk", bufs=1) as qkpool, \
             tc.tile_pool(name="xt", bufs=16) as xtpool, \
             tc.tile_pool(name="pt", bufs=2) as ptpool, \
             tc.tile_pool(name="r1", bufs=4) as r1pool, \
             tc.tile_pool(name="rb", bufs=4) as rbpool, \
             tc.tile_pool(name="ho", bufs=4) as hopool, \
             tc.tile_pool(name="sl", bufs=16) as slpool, \
             tc.tile_pool(name="ob", bufs=4) as obpool, \
             tc.tile_pool(name="pss", bufs=2, space="PSUM") as pss_pool, \
             tc.tile_pool(name="psm", bufs=4, space="PSUM") as psm_pool:

            # ---- constants ----
            wqk_sb = []
            for kc in range(KC):
                t = const.tile([128, 256], BF, tag=f"wqk{kc}")
                nc.sync.dma_start(out=t[:], in_=wqkT[kc * 128:(kc + 1) * 128, :])
                wqk_sb.append(t)
            bq_sb = []
            for fb in range(2):
                t = const.tile([128, 1], F32, tag=f"bq{fb}")
                nc.sync.dma_start(out=t[:], in_=bqk[fb * 128:(fb + 1) * 128, :])
                bq_sb.append(t)
            wout_sb = []
            for fc in range(KC):
                t = const.tile([128, DIM], BF, tag=f"wout{fc}")
                nc.sync.dma_start(out=t[:], in_=woutT[fc * 128:(fc + 1) * 128, :])
                wout_sb.append(t)
            bout_sb = const.tile([128, DIM], F32, tag="bout")
            nc.sync.dma_start(out=bout_sb[:], in_=boutb[:])
            ctx_sb = {}
            for b in range(B):
                for h in range(HPC):
                    t = const.tile([128, NKC * CW], BF, tag=f"ctx{b}{h}")
                    nc.sync.dma_start(out=t[:], in_=ctxa[b, h, :, :])
                    ctx_sb[b, h] = t

            # persistent q^T / k^T, packed per head along partitions
            qT = qkpool.tile([128, BN], BF, tag="qT")
            kT = qkpool.tile([128, BN], BF, tag="kT")

            # ---- phase B: qk projection ----
            for t in range(TC):
                xts = []
                for kc in range(KC):
                    xt = xtpool.tile([128, 512], BF, tag="xt")
                    nc.sync.dma_start(
                        out=xt[:],
                        in_=xT[kc * 128:(kc + 1) * 128, t * 512:(t + 1) * 512])
                    xts.append(xt)
                for fb, dst in ((0, qT), (1, kT)):
                    ps = psm_pool.tile([128, 512], F32, tag="psm")
                    for kc in range(KC):
                        nc.tensor.matmul(
                            ps[:], wqk_sb[kc][:, fb * 128:(fb + 1) * 128],
                            xts[kc][:], start=(kc == 0), stop=(kc == KC - 1))
                    nc.vector.tensor_scalar_add(
                        dst[:, t * 512:(t + 1) * 512], ps[:], bq_sb[fb][:])

            # ---- phase C: attention ----
            for b in range(B):
                for h in range(HPC):
                    hp = h * HD          # partition base of this head
                    for qg in range(2):  # 1024-query groups
                        q0 = b * N + qg * 1024
                        pt = ptpool.tile([128, NKC * 1024], BF, tag="pt")
                        for kc in range(NKC):
                            ps = pss_pool.tile([128, 1024], F32, tag="pss")
                            for hf in range(2):
                                nc.tensor.matmul(
                                    ps[:, hf * 512:(hf + 1) * 512],
                                    kT[hp:hp + HD,
                                       b * N + kc * 128:b * N + (kc + 1) * 128],
                                    qT[hp:hp + HD,
                                       q0 + hf * 512:q0 + (hf + 1) * 512],
                                    start=True, stop=True)
                            nc.scalar.activation(
                                pt[:, kc * 1024:(kc + 1) * 1024], ps[:],
                                mybir.ActivationFunctionType.Exp, scale=SCALE)
                        for qc in range(2):  # 512-query chunks
                            pav = psm_pool.tile([CW, 512], F32, tag="psm")
                            for kc in range(NKC):
                                nc.tensor.matmul(
                                    pav[:],
                                    ctx_sb[b, h][:, kc * CW:(kc + 1) * CW],
                                    pt[:, kc * 1024 + qc * 512:
                                       kc * 1024 + (qc + 1) * 512],
                                    start=(kc == 0), stop=(kc == NKC - 1))
                            r1 = r1pool.tile([1, 512], F32, tag="r1")
                            nc.vector.reciprocal(r1[:], pav[HD:CW, :])
                            rb = rbpool.tile([HD, 512], F32, tag="rb")
                            nc.gpsimd.partition_broadcast(rb[:], r1[:])
                            ho = hopool.tile([HD, 512], BF, tag="ho")
                            nc.vector.tensor_tensor(
                                out=ho[:], in0=pav[0:HD, :], in1=rb[:],
                                op=mybir.AluOpType.mult)
                            g0 = q0 + qc * 512
                            slab, r0 = g0 // SLAB, g0 % SLAB
                            nc.sync.dma_start(
                                out=a2a_in[slab, h * HD:(h + 1) * HD,
                                           r0:r0 + 512],
                                in_=ho[:])

            # ---- phase D: re-shard head-parallel -> row-parallel ----
            nc.gpsimd.collective_compute(
                "AllToAll", mybir.AluOpType.bypass,
                replica_groups=[list(range(NC))],
                ins=[a2a_in.ap().opt()], outs=[a2a_out.ap().opt()])

            # ---- phase E: output projection on local slab ----
            for rc in range(SLAB // 128):
                sls = []
                for fc in range(KC):
                    sl = slpool.tile([128, 128], BF, tag="sl")
                    nc.sync.dma_start(
                        out=sl[:],
                        in_=a2a_out[fc, :, rc * 128:(rc + 1) * 128])
                    sls.append(sl)
                pso = [psm_pool.tile([128, 512], F32, tag="psm",
                                     name=f"pso{rc}_{i}") for i in range(2)]
                for fc in range(KC):
                    for n in range(2):
                        nc.tensor.matmul(
                            pso[n][:], sls[fc][:],
                            wout_sb[fc][:, n * 512:(n + 1) * 512],
                            start=(fc == 0), stop=(fc == KC - 1))
                for n in range(2):
                    ob = obpool.tile([128, 512], F32, tag="ob")
                    nc.vector.tensor_tensor(
                        out=ob[:], in0=pso[n][:],
                        in1=bout_sb[:, n * 512:(n + 1) * 512],
                        op=mybir.AluOpType.add)
                    nc.sync.dma_start(
                        out=out[rc * 128:(rc + 1) * 128,
                                n * 512:(n + 1) * 512],
                        in_=ob[:])
    nc.compile()
    return nc


def prep_inputs(x, context, Wqkv, bqkv, Wout, bout):
    """Host-side sharding: returns in_maps for the 8 cores."""
    x = np.asarray(x, np.float32)
    context = np.asarray(context, np.float32)
    Wqkv = np.asarray(Wqkv, np.float32)
    bqkv = np.asarray(bqkv, np.float32)
    Wout = np.asarray(Wout, np.float32)
    bout = np.asarray(bout, np.float32)

    xT = np.ascontiguousarray(x.reshape(BN, DIM).T).astype(BF16)
    woutT = np.ascontiguousarray(Wout.T).astype(BF16)
    boutb = np.broadcast_to(bout, (128, DIM)).astype(np.float32).copy()

    in_maps = []
    for c in range(NC):
        h0 = c * HPC
        # feature order: [q_h0 | q_h1] then [k_h0 | k_h1]
        wq = Wqkv[h0 * HD:(h0 + HPC) * HD]
        wk = Wqkv[DIM + h0 * HD:DIM + (h0 + HPC) * HD]
        wqkT = np.ascontiguousarray(
            np.concatenate([wq, wk], axis=0).T).astype(BF16)
        bq = np.concatenate([bqkv[h0 * HD:(h0 + HPC) * HD],
                             bqkv[DIM + h0 * HD:DIM + (h0 + HPC) * HD]])
        bq = bq.reshape(2 * 128, 1).astype(np.float32)
        ctxa = np.ones((B, HPC, 128, NKC, CW), np.float32)
        for h in range(HPC):
            g = h0 + h
            arr = context[:, :, g * HD:(g + 1) * HD].reshape(B, NKC, 128, HD)
            ctxa[:, h, :, :, :HD] = arr.transpose(0, 2, 1, 3)
        in_maps.append({
            "xT": xT,
            "wqkT": wqkT,
            "bqk": bq,
            "ctxa": ctxa.reshape(B, HPC, 128, NKC * CW).astype(BF16),
            "woutT": woutT,
            "boutb": boutb,
        })
    return in_maps


_NC_CACHE = None


def _get_nc():
    global _NC_CACHE
    if _NC_CACHE is None:
        _NC_CACHE = build()
    return _NC_CACHE


def run(in_maps, trace=False):
    nc = _get_nc()
    res = run_bass_kernel_spmd(nc, in_maps, core_ids=list(range(NC)),
                               trace=trace)
    slabs = [np.asarray(res.results[c]["out"]) for c in range(NC)]
    full = np.concatenate(slabs, axis=0).reshape(B, N, DIM)
    return full.astype(np.float32), res


def kernel(x, context, Wqkv, bqkv, Wout, bout):
    in_maps = prep_inputs(x, context, Wqkv, bqkv, Wout, bout)
    out, _ = run(in_maps, trace=False)
    return out
